# revision 6
# baseline (speedup 1.0000x reference)
"""Trainium2 Bass kernel for NeighborhoodNormalization.

Math: the reference builds a per-point homogeneous transform
T = [[ux,-uy,0,px],[uy,ux,0,py],[0,0,1,pz],[0,0,0,1]] (u = p/||p||),
inverts it, and applies it to 64 neighbors per point.  With
r2 = px^2+py^2, n = ||p||, a = n/r2, cx = px*a, cy = py*a and
d = q - p (per-neighbor delta), the output reduces to

    out.x =  cx*dx + cy*dy
    out.y = -cy*dx + cx*dy
    out.z =  dz

Pure elementwise math (memory-bound; ~25.4 MB of HBM traffic per core at
a ~430 GB/s shared in+out DMA cap -> ~59 us floor).  Sharding: pure data
parallel over the N=8192 point axis across 8 cores (1024 points/core).

Per-core layout: 16384 points = 128 partitions x 128 columns, partition
p = b*8 + s holds points with local n = s*128 + t (t = column).  Neighbor
rows (64*3 floats) are contiguous in HBM per point, so DMAs move
[128 partitions x 12 KiB contiguous] blocks (16 columns per group).

Compute per group of 16 columns (wide 4D tensor_tensor ops; per-column
coefficients broadcast along K via 0-stride APs):
    SUB: d  = nb - p   (xy only when ACT covers z per-column, else xyz)
    PC:  pC = d_xy * (cx, cy)     -> pairsum gives out.x
    PD:  pD = d_xy * (-cy, cx)    -> pairsum gives out.y
    SX:  ot.x = pC[0] + pC[1]
    SY:  ot.y = pD[0] + pD[1]

Each group is owned end-to-end by ONE compute engine (V owns 5 groups,
GPS owns 3) so V and GPS never touch the same tiles concurrently --
cross-engine interleaved writes to shared SBUF rows were measured to
slow wide DVE ops up to 2.5x.  ACT does the z-subtraction per-column on
6 groups and issues the output DMAs.  Busy: ~51 us V / ~52 us GPS /
~43 us ACT, all under the ~59 us DMA floor.
"""

import sys

if "/opt/trn_rl_repo" not in sys.path:
    sys.path.insert(0, "/opt/trn_rl_repo")

import numpy as np

import concourse.bass as bass
import concourse.bacc as bacc
import concourse.mybir as mybir
from concourse.tile import TileContext
from concourse.bass_utils import run_bass_kernel_spmd

B = 16
N = 8192
K = 64
NCORES = 8
NLOC = N // NCORES  # 1024 points per core
P = 128             # SBUF partitions
S = NLOC // P       # 8 partition sub-blocks per batch entry
T = (B * NLOC) // P  # 128 point-columns per partition
G = 16              # columns per DMA group
NG = T // G

F32 = mybir.dt.float32
F16 = mybir.dt.float16
OP = mybir.AluOpType
AF = mybir.ActivationFunctionType

# z-subtraction runs as a wide op on GPS for every group (V carries the
# xy pipeline; ACT only issues output DMAs).
ACT_Z_GROUPS = frozenset()

_CACHE = {}


def _build_nc():
    nc = bacc.Bacc(None, target_bir_lowering=False)

    pts = nc.declare_dram_parameter("points", [B, NLOC, 3], F32, isOutput=False)
    nb = nc.declare_dram_parameter("neighborhoods", [B, NLOC, K, 3], F32, isOutput=False)
    out = nc.declare_dram_parameter("out", [B, NLOC, K, 3], F32, isOutput=True)

    # partition = (b s), columns = t, free = 192 floats per point
    nbr = nb[:].rearrange("b (s t) k c -> (b s) t (k c)", s=S)
    outr = out[:].rearrange("b (s t) k c -> (b s) t (k c)", s=S)
    ptsr = pts[:].rearrange("b (s t) c -> (b s) (t c)", s=S)

    with TileContext(nc) as tc:
        with tc.tile_pool(name="const", bufs=1) as cpool, \
             tc.tile_pool(name="io_in", bufs=4) as inpool, \
             tc.tile_pool(name="io_out", bufs=4) as outpool, \
             tc.tile_pool(name="prod", bufs=3) as ppool:

            pts_sb = cpool.tile([P, T, 3], F32, tag="pts")
            nc.sync.dma_start(
                out=pts_sb[:].rearrange("p t c -> p (t c)"), in_=ptsr)
            px = pts_sb[:, :, 0]
            py = pts_sb[:, :, 1]
            pz = pts_sb[:, :, 2]

            def ctile(tag):
                return cpool.tile([P, T], F32, tag=tag, name=tag)

            t1 = ctile("t1")
            t2 = ctile("t2")
            r2 = ctile("r2")
            n2 = ctile("n2")
            nn = ctile("nn")
            ir2 = ctile("ir2")
            aa = ctile("aa")
            npz = ctile("npz")
            cA = cpool.tile([P, T, 2], F32, tag="cA", name="cA")  # (cx,  cy)
            cD = cpool.tile([P, T, 2], F32, tag="cD", name="cD")  # (-cy, cx)
            cA16 = cpool.tile([P, T, 2], F16, tag="cA16", name="cA16")
            cD16 = cpool.tile([P, T, 2], F16, tag="cD16", name="cD16")

            nc.vector.tensor_mul(out=t1[:], in0=px, in1=px)
            nc.vector.tensor_mul(out=t2[:], in0=py, in1=py)
            nc.vector.tensor_add(out=r2[:], in0=t1[:], in1=t2[:])
            nc.vector.tensor_mul(out=t1[:], in0=pz, in1=pz)
            nc.vector.tensor_add(out=n2[:], in0=r2[:], in1=t1[:])
            nc.scalar.sqrt(out=nn[:], in_=n2[:])
            nc.vector.reciprocal(out=ir2[:], in_=r2[:])
            nc.vector.tensor_mul(out=aa[:], in0=nn[:], in1=ir2[:])
            # cA = (cx, cy) = (px*a, py*a); cD = (-cy, cx)
            nc.vector.tensor_mul(out=cA[:, :, 0], in0=px, in1=aa[:])
            nc.vector.tensor_mul(out=cA[:, :, 1], in0=py, in1=aa[:])
            nc.vector.scalar_tensor_tensor(
                out=cD[:, :, 0], in0=py, scalar=-1.0, in1=aa[:],
                op0=OP.mult, op1=OP.mult)
            nc.vector.tensor_mul(out=cD[:, :, 1], in0=px, in1=aa[:])
            nc.gpsimd.tensor_scalar(
                out=npz[:], in0=pz, scalar1=-1.0, scalar2=None, op0=OP.mult)
            # fp16 copies of the coefficient pair tiles
            nc.scalar.copy(
                out=cA16[:].rearrange("p t c -> p (t c)"),
                in_=cA[:].rearrange("p t c -> p (t c)"))
            nc.scalar.copy(
                out=cD16[:].rearrange("p t c -> p (t c)"),
                in_=cD[:].rearrange("p t c -> p (t c)"))

            for g in range(NG):
                gs, ge = g * G, (g + 1) * G

                nb_t = inpool.tile([P, G, K, 3], F32, tag="nb", name=f"nb{g}")
                nc.sync.dma_start(
                    out=nb_t[:].rearrange("p g k c -> p g (k c)"),
                    in_=nbr[:, gs:ge, :],
                )
                ot = outpool.tile([P, G, K, 3], F32, tag="ot", name=f"ot{g}")
                d16 = ppool.tile([P, G, K, 2], F16, tag="d16", name=f"d16_{g}")
                # planar product tiles: [P, c, G, K] so pair-sums read
                # contiguous fp16 planes
                pC = ppool.tile([P, 2, G, K], F16, tag="pC", name=f"pC{g}")
                pD = ppool.tile([P, 2, G, K], F16, tag="pD", name=f"pD{g}")
                pCv = pC[:].rearrange("p c g k -> p g k c")
                pDv = pD[:].rearrange("p c g k -> p g k c")

                bp_xy = pts_sb[:, gs:ge, None, 0:2].broadcast_to([P, G, K, 2])
                bcA = cA16[:, gs:ge, None, :].broadcast_to([P, G, K, 2])
                bcD = cD16[:, gs:ge, None, :].broadcast_to([P, G, K, 2])

                # d_xy (fp16) wide on V; z wide on GPS
                nc.vector.tensor_sub(out=d16[:], in0=nb_t[:, :, :, 0:2],
                                     in1=bp_xy)
                nc.gpsimd.tensor_sub(
                    out=ot[:, :, :, 2], in0=nb_t[:, :, :, 2],
                    in1=pts_sb[:, gs:ge, 2, None].broadcast_to([P, G, K]))

                # pC = (cx*dx, cy*dy); pD = (-cy*dx, cx*dy)  (all-fp16, 2x mode)
                nc.vector.tensor_mul(out=pCv, in0=d16[:], in1=bcA)
                nc.vector.tensor_mul(out=pDv, in0=d16[:], in1=bcD)
                # pairsums -> rotated xy (contiguous fp16 planes in, fp32 out)
                nc.vector.tensor_add(
                    out=ot[:, :, :, 0], in0=pC[:, 0], in1=pC[:, 1])
                nc.vector.tensor_add(
                    out=ot[:, :, :, 1], in0=pD[:, 0], in1=pD[:, 1])

                # out-DMA on the ACT HWDGE ring so it overlaps the SP-ring
                # input stream (HWDGE is FIFO per issuing engine).
                nc.scalar.dma_start(
                    out=outr[:, gs:ge, :],
                    in_=ot[:].rearrange("p g k c -> p g (k c)"),
                )

    nc.compile()
    return nc


def _get_nc():
    if "nc" not in _CACHE:
        _CACHE["nc"] = _build_nc()
    return _CACHE["nc"]


def kernel(points, neighborhoods):
    pts = np.ascontiguousarray(np.asarray(points, dtype=np.float32))
    nb = np.ascontiguousarray(np.asarray(neighborhoods, dtype=np.float32))
    assert pts.shape == (B, N, 3), pts.shape
    assert nb.shape == (B, N, K, 3), nb.shape

    in_maps = []
    for c in range(NCORES):
        sl = slice(c * NLOC, (c + 1) * NLOC)
        in_maps.append({
            "points": np.ascontiguousarray(pts[:, sl]),
            "neighborhoods": np.ascontiguousarray(nb[:, sl]),
        })

    res = run_bass_kernel_spmd(_get_nc(), in_maps, list(range(NCORES))).results
    out = np.concatenate([res[c]["out"] for c in range(NCORES)], axis=1)
    return out


# revision 7
# speedup vs baseline: 2.2482x; 2.2482x over previous
"""Trainium2 Bass kernel for NeighborhoodNormalization.

Math: the reference builds a per-point homogeneous transform
T = [[ux,-uy,0,px],[uy,ux,0,py],[0,0,1,pz],[0,0,0,1]] (u = p/||p||),
inverts it, and applies it to 64 neighbors per point.  With
r2 = px^2+py^2, n = ||p||, a = n/r2, cx = px*a, cy = py*a and
d = q - p (per-neighbor delta), the output reduces to

    out.x =  cx*dx + cy*dy
    out.y = -cy*dx + cx*dy
    out.z =  dz

Pure elementwise math (memory-bound; ~25.4 MB of HBM traffic per core at
a ~430 GB/s shared in+out DMA cap -> ~59 us floor).  Sharding: pure data
parallel over the N=8192 point axis across 8 cores (1024 points/core).

Per-core layout: 16384 points = 128 partitions x 128 columns, partition
p = b*8 + s holds points with local n = s*128 + t (t = column).  Neighbor
rows (64*3 floats) are contiguous in HBM per point, so DMAs move
[128 partitions x 12 KiB contiguous] blocks (16 columns per group).

Compute per group of 16 columns (wide 4D tensor_tensor ops; per-column
coefficients broadcast along K via 0-stride APs):
    SUB: d  = nb - p   (xy only when ACT covers z per-column, else xyz)
    PC:  pC = d_xy * (cx, cy)     -> pairsum gives out.x
    PD:  pD = d_xy * (-cy, cx)    -> pairsum gives out.y
    SX:  ot.x = pC[0] + pC[1]
    SY:  ot.y = pD[0] + pD[1]

Each group is owned end-to-end by ONE compute engine (V owns 5 groups,
GPS owns 3) so V and GPS never touch the same tiles concurrently --
cross-engine interleaved writes to shared SBUF rows were measured to
slow wide DVE ops up to 2.5x.  ACT does the z-subtraction per-column on
6 groups and issues the output DMAs.  Busy: ~51 us V / ~52 us GPS /
~43 us ACT, all under the ~59 us DMA floor.
"""

import sys

if "/opt/trn_rl_repo" not in sys.path:
    sys.path.insert(0, "/opt/trn_rl_repo")

import numpy as np

import concourse.bass as bass
import concourse.bacc as bacc
import concourse.mybir as mybir
from concourse.tile import TileContext
from concourse.bass_utils import run_bass_kernel_spmd

B = 16
N = 8192
K = 64
NCORES = 8
NLOC = N // NCORES  # 1024 points per core
P = 128             # SBUF partitions
S = NLOC // P       # 8 partition sub-blocks per batch entry
T = (B * NLOC) // P  # 128 point-columns per partition
G = 16              # columns per DMA group
NG = T // G

F32 = mybir.dt.float32
F16 = mybir.dt.float16
OP = mybir.AluOpType
AF = mybir.ActivationFunctionType

# z-subtraction runs as a wide op on GPS for every group (V carries the
# xy pipeline; ACT only issues output DMAs).
ACT_Z_GROUPS = frozenset()

_CACHE = {}


def _build_nc():
    nc = bacc.Bacc(None, target_bir_lowering=False)

    pts = nc.declare_dram_parameter("points", [B, NLOC, 3], F32, isOutput=False)
    nb = nc.declare_dram_parameter("neighborhoods", [B, NLOC, K, 3], F32, isOutput=False)
    out = nc.declare_dram_parameter("out", [B, NLOC, K, 3], F32, isOutput=True)

    # partition = (b s), columns = t, free = 192 floats per point
    nbr = nb[:].rearrange("b (s t) k c -> (b s) t (k c)", s=S)
    outr = out[:].rearrange("b (s t) k c -> (b s) t (k c)", s=S)
    ptsr = pts[:].rearrange("b (s t) c -> (b s) (t c)", s=S)

    with TileContext(nc) as tc:
        with tc.tile_pool(name="const", bufs=1) as cpool, \
             tc.tile_pool(name="io_in", bufs=4) as inpool, \
             tc.tile_pool(name="io_out", bufs=4) as outpool, \
             tc.tile_pool(name="prod", bufs=3) as ppool:

            pts_sb = cpool.tile([P, T, 3], F32, tag="pts")
            nc.sync.dma_start(
                out=pts_sb[:].rearrange("p t c -> p (t c)"), in_=ptsr)
            px = pts_sb[:, :, 0]
            py = pts_sb[:, :, 1]
            pz = pts_sb[:, :, 2]

            def ctile(tag):
                return cpool.tile([P, T], F32, tag=tag, name=tag)

            t1 = ctile("t1")
            t2 = ctile("t2")
            r2 = ctile("r2")
            n2 = ctile("n2")
            nn = ctile("nn")
            ir2 = ctile("ir2")
            aa = ctile("aa")
            npz = ctile("npz")
            cA = cpool.tile([P, T, 2], F32, tag="cA", name="cA")  # (cx,  cy)
            cD = cpool.tile([P, T, 2], F32, tag="cD", name="cD")  # (-cy, cx)
            cA16 = cpool.tile([P, T, 2], F16, tag="cA16", name="cA16")
            cD16 = cpool.tile([P, T, 2], F16, tag="cD16", name="cD16")

            nc.vector.tensor_mul(out=t1[:], in0=px, in1=px)
            nc.vector.tensor_mul(out=t2[:], in0=py, in1=py)
            nc.vector.tensor_add(out=r2[:], in0=t1[:], in1=t2[:])
            nc.vector.tensor_mul(out=t1[:], in0=pz, in1=pz)
            nc.vector.tensor_add(out=n2[:], in0=r2[:], in1=t1[:])
            nc.scalar.sqrt(out=nn[:], in_=n2[:])
            nc.vector.reciprocal(out=ir2[:], in_=r2[:])
            nc.vector.tensor_mul(out=aa[:], in0=nn[:], in1=ir2[:])
            # cA = (cx, cy) = (px*a, py*a); cD = (-cy, cx)
            nc.vector.tensor_mul(out=cA[:, :, 0], in0=px, in1=aa[:])
            nc.vector.tensor_mul(out=cA[:, :, 1], in0=py, in1=aa[:])
            nc.vector.scalar_tensor_tensor(
                out=cD[:, :, 0], in0=py, scalar=-1.0, in1=aa[:],
                op0=OP.mult, op1=OP.mult)
            nc.vector.tensor_mul(out=cD[:, :, 1], in0=px, in1=aa[:])
            nc.gpsimd.tensor_scalar(
                out=npz[:], in0=pz, scalar1=-1.0, scalar2=None, op0=OP.mult)
            # fp16 copies of the coefficient pair tiles
            nc.scalar.copy(
                out=cA16[:].rearrange("p t c -> p (t c)"),
                in_=cA[:].rearrange("p t c -> p (t c)"))
            nc.scalar.copy(
                out=cD16[:].rearrange("p t c -> p (t c)"),
                in_=cD[:].rearrange("p t c -> p (t c)"))

            for g in range(NG):
                gs, ge = g * G, (g + 1) * G

                nb_t = inpool.tile([P, G, K, 3], F32, tag="nb", name=f"nb{g}")
                nc.sync.dma_start(
                    out=nb_t[:].rearrange("p g k c -> p g (k c)"),
                    in_=nbr[:, gs:ge, :],
                )
                ot = outpool.tile([P, G, K, 3], F32, tag="ot", name=f"ot{g}")
                d16 = ppool.tile([P, G, K, 2], F16, tag="d16", name=f"d16_{g}")
                pC = ppool.tile([P, G, K, 2], F16, tag="pC", name=f"pC{g}")
                pD = ppool.tile([P, G, K, 2], F16, tag="pD", name=f"pD{g}")

                bp_xy = pts_sb[:, gs:ge, None, 0:2].broadcast_to([P, G, K, 2])
                bcA = cA16[:, gs:ge, None, :].broadcast_to([P, G, K, 2])
                bcD = cD16[:, gs:ge, None, :].broadcast_to([P, G, K, 2])

                # d_xy (fp16) wide on V; z wide on GPS
                nc.vector.tensor_sub(out=d16[:], in0=nb_t[:, :, :, 0:2],
                                     in1=bp_xy)
                nc.gpsimd.tensor_sub(
                    out=ot[:, :, :, 2], in0=nb_t[:, :, :, 2],
                    in1=pts_sb[:, gs:ge, 2, None].broadcast_to([P, G, K]))

                # pC = (cx*dx, cy*dy); pD = (-cy*dx, cx*dy)  (all-fp16, 2x mode)
                nc.vector.tensor_mul(out=pC[:], in0=d16[:], in1=bcA)
                nc.vector.tensor_mul(out=pD[:], in0=d16[:], in1=bcD)
                # pairsums -> rotated xy (fp16 in, fp32 out)
                nc.vector.tensor_add(
                    out=ot[:, :, :, 0], in0=pC[:, :, :, 0], in1=pC[:, :, :, 1])
                nc.vector.tensor_add(
                    out=ot[:, :, :, 1], in0=pD[:, :, :, 0], in1=pD[:, :, :, 1])

                # out-DMA on the ACT HWDGE ring so it overlaps the SP-ring
                # input stream (HWDGE is FIFO per issuing engine).
                nc.scalar.dma_start(
                    out=outr[:, gs:ge, :],
                    in_=ot[:].rearrange("p g k c -> p g (k c)"),
                )

    nc.compile()
    return nc


def _get_nc():
    if "nc" not in _CACHE:
        _CACHE["nc"] = _build_nc()
    return _CACHE["nc"]


def kernel(points, neighborhoods):
    pts = np.ascontiguousarray(np.asarray(points, dtype=np.float32))
    nb = np.ascontiguousarray(np.asarray(neighborhoods, dtype=np.float32))
    assert pts.shape == (B, N, 3), pts.shape
    assert nb.shape == (B, N, K, 3), nb.shape

    in_maps = []
    for c in range(NCORES):
        sl = slice(c * NLOC, (c + 1) * NLOC)
        in_maps.append({
            "points": np.ascontiguousarray(pts[:, sl]),
            "neighborhoods": np.ascontiguousarray(nb[:, sl]),
        })

    res = run_bass_kernel_spmd(_get_nc(), in_maps, list(range(NCORES))).results
    out = np.concatenate([res[c]["out"] for c in range(NCORES)], axis=1)
    return out


# revision 8
# speedup vs baseline: 2.8369x; 1.2618x over previous
"""Trainium2 Bass kernel for NeighborhoodNormalization.

Math: the reference builds a per-point homogeneous transform
T = [[ux,-uy,0,px],[uy,ux,0,py],[0,0,1,pz],[0,0,0,1]] (u = p/||p||),
inverts it, and applies it to 64 neighbors per point.  With
r2 = px^2+py^2, n = ||p||, a = n/r2, cx = px*a, cy = py*a and
d = q - p (per-neighbor delta), the output reduces to

    out.x =  cx*dx + cy*dy
    out.y = -cy*dx + cx*dy
    out.z =  dz

Pure elementwise math (memory-bound; ~25.4 MB of HBM traffic per core at
a ~430 GB/s shared in+out DMA cap -> ~59 us floor).  Sharding: pure data
parallel over the N=8192 point axis across 8 cores (1024 points/core).

Per-core layout: 16384 points = 128 partitions x 128 columns, partition
p = b*8 + s holds points with local n = s*128 + t (t = column).  Neighbor
rows (64*3 floats) are contiguous in HBM per point, so DMAs move
[128 partitions x 12 KiB contiguous] blocks (16 columns per group).

Compute per group of 16 columns (wide 4D tensor_tensor ops; per-column
coefficients broadcast along K via 0-stride APs):
    SUB: d  = nb - p   (xy only when ACT covers z per-column, else xyz)
    PC:  pC = d_xy * (cx, cy)     -> pairsum gives out.x
    PD:  pD = d_xy * (-cy, cx)    -> pairsum gives out.y
    SX:  ot.x = pC[0] + pC[1]
    SY:  ot.y = pD[0] + pD[1]

Each group is owned end-to-end by ONE compute engine (V owns 5 groups,
GPS owns 3) so V and GPS never touch the same tiles concurrently --
cross-engine interleaved writes to shared SBUF rows were measured to
slow wide DVE ops up to 2.5x.  ACT does the z-subtraction per-column on
6 groups and issues the output DMAs.  Busy: ~51 us V / ~52 us GPS /
~43 us ACT, all under the ~59 us DMA floor.
"""

import sys

if "/opt/trn_rl_repo" not in sys.path:
    sys.path.insert(0, "/opt/trn_rl_repo")

import numpy as np

import concourse.bass as bass
import concourse.bacc as bacc
import concourse.mybir as mybir
from concourse.tile import TileContext
from concourse.bass_utils import run_bass_kernel_spmd

B = 16
N = 8192
K = 64
NCORES = 8
NLOC = N // NCORES  # 1024 points per core
P = 128             # SBUF partitions
S = NLOC // P       # 8 partition sub-blocks per batch entry
T = (B * NLOC) // P  # 128 point-columns per partition
G = 16              # columns per DMA group
NG = T // G

F32 = mybir.dt.float32
F16 = mybir.dt.float16
OP = mybir.AluOpType
AF = mybir.ActivationFunctionType

# z-subtraction runs per-column on ACT for every group (V carries the
# xy pipeline; GPS idles -- concurrent GPS wide ops were measured to
# inflate V op durations ~1.3-1.6x via SBUF contention).
ACT_Z_GROUPS = frozenset(range(NG))

_CACHE = {}


def _build_nc():
    nc = bacc.Bacc(None, target_bir_lowering=False)

    pts = nc.declare_dram_parameter("points", [B, NLOC, 3], F32, isOutput=False)
    nb = nc.declare_dram_parameter("neighborhoods", [B, NLOC, K, 3], F32, isOutput=False)
    out = nc.declare_dram_parameter("out", [B, NLOC, K, 3], F32, isOutput=True)

    # partition = (b s), columns = t, free = 192 floats per point
    nbr = nb[:].rearrange("b (s t) k c -> (b s) t (k c)", s=S)
    outr = out[:].rearrange("b (s t) k c -> (b s) t (k c)", s=S)
    ptsr = pts[:].rearrange("b (s t) c -> (b s) (t c)", s=S)

    with TileContext(nc) as tc:
        with tc.tile_pool(name="const", bufs=1) as cpool, \
             tc.tile_pool(name="io_in", bufs=4) as inpool, \
             tc.tile_pool(name="io_out", bufs=4) as outpool, \
             tc.tile_pool(name="prod", bufs=3) as ppool:

            pts_sb = cpool.tile([P, T, 3], F32, tag="pts")
            nc.sync.dma_start(
                out=pts_sb[:].rearrange("p t c -> p (t c)"), in_=ptsr)
            px = pts_sb[:, :, 0]
            py = pts_sb[:, :, 1]
            pz = pts_sb[:, :, 2]

            def ctile(tag):
                return cpool.tile([P, T], F32, tag=tag, name=tag)

            t1 = ctile("t1")
            t2 = ctile("t2")
            r2 = ctile("r2")
            n2 = ctile("n2")
            nn = ctile("nn")
            ir2 = ctile("ir2")
            aa = ctile("aa")
            npz = ctile("npz")
            cA = cpool.tile([P, T, 2], F32, tag="cA", name="cA")  # (cx,  cy)
            cD = cpool.tile([P, T, 2], F32, tag="cD", name="cD")  # (-cy, cx)
            cA16 = cpool.tile([P, T, 2], F16, tag="cA16", name="cA16")
            cD16 = cpool.tile([P, T, 2], F16, tag="cD16", name="cD16")

            nc.vector.tensor_mul(out=t1[:], in0=px, in1=px)
            nc.vector.tensor_mul(out=t2[:], in0=py, in1=py)
            nc.vector.tensor_add(out=r2[:], in0=t1[:], in1=t2[:])
            nc.vector.tensor_mul(out=t1[:], in0=pz, in1=pz)
            nc.vector.tensor_add(out=n2[:], in0=r2[:], in1=t1[:])
            nc.scalar.sqrt(out=nn[:], in_=n2[:])
            nc.vector.reciprocal(out=ir2[:], in_=r2[:])
            nc.vector.tensor_mul(out=aa[:], in0=nn[:], in1=ir2[:])
            # cA = (cx, cy) = (px*a, py*a); cD = (-cy, cx)
            nc.vector.tensor_mul(out=cA[:, :, 0], in0=px, in1=aa[:])
            nc.vector.tensor_mul(out=cA[:, :, 1], in0=py, in1=aa[:])
            nc.vector.scalar_tensor_tensor(
                out=cD[:, :, 0], in0=py, scalar=-1.0, in1=aa[:],
                op0=OP.mult, op1=OP.mult)
            nc.vector.tensor_mul(out=cD[:, :, 1], in0=px, in1=aa[:])
            nc.gpsimd.tensor_scalar(
                out=npz[:], in0=pz, scalar1=-1.0, scalar2=None, op0=OP.mult)
            # fp16 copies of the coefficient pair tiles
            nc.scalar.copy(
                out=cA16[:].rearrange("p t c -> p (t c)"),
                in_=cA[:].rearrange("p t c -> p (t c)"))
            nc.scalar.copy(
                out=cD16[:].rearrange("p t c -> p (t c)"),
                in_=cD[:].rearrange("p t c -> p (t c)"))

            for g in range(NG):
                gs, ge = g * G, (g + 1) * G

                nb_t = inpool.tile([P, G, K, 3], F32, tag="nb", name=f"nb{g}")
                nc.sync.dma_start(
                    out=nb_t[:].rearrange("p g k c -> p g (k c)"),
                    in_=nbr[:, gs:ge, :],
                )
                ot = outpool.tile([P, G, K, 3], F32, tag="ot", name=f"ot{g}")
                d16 = ppool.tile([P, G, K, 2], F16, tag="d16", name=f"d16_{g}")
                # product quad (cx*dx, cy*dy, -cy*dx, cx*dy): both pair-sums
                # then collapse into ONE 2048-wide add
                Q = ppool.tile([P, G, K, 4], F16, tag="Q", name=f"Q{g}")
                Q5 = Q[:].rearrange("p g k (two c) -> p g k two c", two=2)

                bp_xy = pts_sb[:, gs:ge, None, 0:2].broadcast_to([P, G, K, 2])
                bcA = cA16[:, gs:ge, None, :].broadcast_to([P, G, K, 2])
                bcD = cD16[:, gs:ge, None, :].broadcast_to([P, G, K, 2])

                # d_xy (fp16) wide on V; z per-column on ACT
                nc.vector.tensor_sub(out=d16[:], in0=nb_t[:, :, :, 0:2],
                                     in1=bp_xy)
                for i in range(G):
                    t = gs + i
                    nc.scalar.activation(
                        out=ot[:, i, :, 2], in_=nb_t[:, i, :, 2],
                        func=AF.Identity, bias=npz[:, t:t + 1], scale=1.0)

                # Q[0:2] = (cx*dx, cy*dy); Q[2:4] = (-cy*dx, cx*dy)  (2x mode)
                nc.vector.tensor_mul(out=Q[:, :, :, 0:2], in0=d16[:], in1=bcA)
                nc.vector.tensor_mul(out=Q[:, :, :, 2:4], in0=d16[:], in1=bcD)
                # both pair-sums in one 2048-wide add -> rotated xy
                nc.vector.tensor_add(
                    out=ot[:, :, :, 0:2], in0=Q5[:, :, :, :, 0],
                    in1=Q5[:, :, :, :, 1])

                # out-DMA on the ACT HWDGE ring so it overlaps the SP-ring
                # input stream (HWDGE is FIFO per issuing engine).
                nc.scalar.dma_start(
                    out=outr[:, gs:ge, :],
                    in_=ot[:].rearrange("p g k c -> p g (k c)"),
                )

    nc.compile()
    return nc


def _get_nc():
    if "nc" not in _CACHE:
        _CACHE["nc"] = _build_nc()
    return _CACHE["nc"]


def kernel(points, neighborhoods):
    pts = np.ascontiguousarray(np.asarray(points, dtype=np.float32))
    nb = np.ascontiguousarray(np.asarray(neighborhoods, dtype=np.float32))
    assert pts.shape == (B, N, 3), pts.shape
    assert nb.shape == (B, N, K, 3), nb.shape

    in_maps = []
    for c in range(NCORES):
        sl = slice(c * NLOC, (c + 1) * NLOC)
        in_maps.append({
            "points": np.ascontiguousarray(pts[:, sl]),
            "neighborhoods": np.ascontiguousarray(nb[:, sl]),
        })

    res = run_bass_kernel_spmd(_get_nc(), in_maps, list(range(NCORES))).results
    out = np.concatenate([res[c]["out"] for c in range(NCORES)], axis=1)
    return out
